# revision 23
# baseline (speedup 1.0000x reference)
"""Trainium2 Bass kernel: masked multi-head self-attention block.

out = softmax_mask((x @ Wq) (x @ Wk)^T / sqrt(d)) (x @ Wv) @ Wp + b

Sharding: data-parallel over batch B=8 across the 8 NeuronCores (one
batch row per core); weights replicated; no collectives.

Key compaction: masked keys contribute exactly zero, so each core
gathers only the valid key rows of x (padded to a 128 multiple; padded
slots get a -1e30 score bias -> exp = 0). K/V and attention run on
NK ~= 1152 keys instead of 2048.

v4 structure:
  - Prelude (PE): gathered-chunk transposes -> K^T[0] -> V -> full-x
    transposes -> Q^T[0]; attention starts immediately after (~70us).
    K^T[1..5] and Q^T[1..5] are computed *during* the attention stream
    from a backlog (2 small matmul batches per step) using the proj
    PSUM slot, which is free until the first proj chunks arrive.
  - DMA queues split: x loads on sync+gpsimd, weights on scalar,
    gathers first on gpsimd.
  - Attention: one flat software-pipelined stream over (qblock,
    headpair, keychunk) steps with S-lookahead 2 crossing boundaries;
    merged exp [128,2,512] on ScalarE; merged running-sum on DVE;
    col-packed PV pair; denominator matmuls after the next S pair;
    reciprocal after broadcast. proj drains from the same backlog; the
    last qblock's proj alternates two PSUM tags so the tail pipelines.
"""
import numpy as np

import concourse.bass as bass
import concourse.tile as tile
from concourse import bacc, mybir
from concourse.bass_utils import run_bass_kernel_spmd
from concourse.masks import make_identity

F32 = mybir.dt.float32
BF16 = mybir.dt.bfloat16
I32 = mybir.dt.int32

B, N, DIM = 8, 2048, 768
H, D = 12, 64
SCALE = D ** -0.5
NCH = N // 128        # 16 token chunks
KCH = DIM // 128      # 6 feature chunks
QH = 4                # query blocks
QW = N // QH          # 512 queries per block
Exp = mybir.ActivationFunctionType.Exp


def _nslices(w):
    out = [512] * (w // 512)
    if w % 512:
        out.append(w % 512)
    return out


def _build(nc, tc, aps, nkc):
    x_d, ki_d, kb_d, wqkv_d, wp_d, bp_d, o_d = aps
    NK = nkc * 128

    cpool = tc.alloc_tile_pool(name="const", bufs=1)
    ident = cpool.tile([128, 128], BF16)
    make_identity(nc, ident)
    ones_c = cpool.tile([128, 128], BF16)
    nc.vector.memset(ones_c, 0.0)
    nc.vector.memset(ones_c[:, 0:1], 1.0)
    kb_t = cpool.tile([128, nkc], F32)
    nc.sync.dma_start(out=kb_t, in_=kb_d.rearrange("(j p) -> p j", p=128))
    ki_t = cpool.tile([128, nkc], I32)
    nc.gpsimd.dma_start(out=ki_t, in_=ki_d.rearrange("(j p) -> p j", p=128))
    bp_bc = cpool.tile([128, DIM], F32)
    bp_ap = bass.AP(tensor=bp_d.tensor, offset=bp_d.offset,
                    ap=[[0, 128], list(bp_d.ap[0])])
    nc.sync.dma_start(out=bp_bc, in_=bp_ap)

    # persistent tiles (live through attention; released at the end)
    qkvpool = tc.alloc_tile_pool(name="qkv_sb", bufs=1)
    qt = [qkvpool.tile([128, N], BF16, tag=f"qt{m}", name=f"qt{m}")
          for m in range(KCH)]
    kt = [qkvpool.tile([128, NK], BF16, tag=f"kt{m}", name=f"kt{m}")
          for m in range(KCH)]
    v_nat = [qkvpool.tile([128, DIM], BF16, tag=f"vn{t}", name=f"vn{t}")
             for t in range(nkc)]
    wp_sb = [qkvpool.tile([128, DIM], BF16, tag=f"wp{c}", name=f"wp{c}")
             for c in range(KCH)]
    ot = [[qkvpool.tile([128, QW], BF16, tag=f"ot{c}_{q}", name=f"ot{c}_{q}")
           for q in range(QH)] for c in range(KCH)]
    xt = [qkvpool.tile([128, N], BF16, tag=f"xt{c}", name=f"xt{c}")
          for c in range(KCH)]
    xct = [qkvpool.tile([128, NK], BF16, tag=f"xct{c}", name=f"xct{c}")
           for c in range(KCH)]
    wq_sb = [qkvpool.tile([128, 3 * DIM], BF16, tag=f"wq{c}", name=f"wq{c}")
             for c in range(KCH)]

    with tc.tile_pool(name="stage_sb", bufs=3) as spool:
        # gathers first on gpsimd queue; x loads split sync/gpsimd;
        # weights on scalar queue
        gath = []
        for t in range(nkc):
            xg = spool.tile([128, DIM], F32, tag="xg", name=f"xg{t}",
                            bufs=3)
            nc.gpsimd.indirect_dma_start(
                out=xg, out_offset=None, in_=x_d,
                in_offset=bass.IndirectOffsetOnAxis(
                    ap=ki_t[:, t:t + 1], axis=0))
            gath.append(xg)
        wstage = []
        for c in range(KCH):
            ws = spool.tile([128, 3 * DIM], F32, tag="wstage",
                            name=f"ws{c}", bufs=2)
            nc.gpsimd.dma_start(out=ws,
                                in_=wqkv_d[c * 128:(c + 1) * 128, :])
            wstage.append(ws)
        loads = []
        for ti in range(NCH):
            xs = spool.tile([128, DIM], F32, tag="xs", name=f"xs{ti}",
                            bufs=3)
            nc.sync.dma_start(out=xs, in_=x_d[ti * 128:(ti + 1) * 128, :])
            loads.append(xs)

        with tc.tile_pool(name="ps_a", bufs=4, space="PSUM") as ps_a:

            def transpose_chunk(src, dst, col, i):
                xb = spool.tile([128, DIM], BF16, tag="xb", name=f"xb{i}",
                                bufs=3)
                nc.scalar.copy(xb, src)
                for c in range(KCH):
                    tp = ps_a.tile([128, 128], BF16, tag="tp", name="tp")
                    nc.tensor.transpose(
                        tp, xb[:, c * 128:(c + 1) * 128], ident)
                    nc.vector.tensor_copy(
                        dst[c][:, col * 128:(col + 1) * 128], tp)

            # gathered chunks -> xct; weight casts interleaved on DVE
            for t in range(nkc):
                transpose_chunk(gath[t], xct, t, t)
                if 2 <= t < 2 + KCH:
                    nc.vector.tensor_copy(wq_sb[t - 2], wstage[t - 2])
            for c in range(max(0, nkc - 2), KCH):
                nc.vector.tensor_copy(wq_sb[c], wstage[c])
            # full x chunks -> xt (keeps PE busy during weight loads)
            for ti in range(NCH):
                transpose_chunk(loads[ti], xt, ti, nkc + ti)

            # K^T chunk 0 (needed by the first attention steps)
            with tc.tile_pool(name="ps_k", bufs=1, space="PSUM") as ps_k:
                mm_ps = ps_k.tile([128, NK], F32, tag="k_ps", name="k_ps")
                for c in range(KCH):
                    off = 0
                    for w in _nslices(NK):
                        nc.tensor.matmul(
                            mm_ps[:, off:off + w],
                            wq_sb[c][:, DIM:DIM + 128],
                            xct[c][:, off:off + w],
                            start=(c == 0), stop=(c == KCH - 1))
                        off += w
                nc.scalar.copy(kt[0], mm_ps)
            # V
            with tc.tile_pool(name="ps_v", bufs=2, space="PSUM") as ps_v:
                for t in range(nkc):
                    v_ps = ps_v.tile([128, 2, 512], F32, tag="v_ps",
                                     name="v_ps")
                    for c in range(KCH):
                        nc.tensor.matmul(
                            v_ps[:, 0, :],
                            xct[c][:, t * 128:(t + 1) * 128],
                            wq_sb[c][:, 2 * DIM:2 * DIM + 512],
                            start=(c == 0), stop=(c == KCH - 1))
                        nc.tensor.matmul(
                            v_ps[:, 1, 0:256],
                            xct[c][:, t * 128:(t + 1) * 128],
                            wq_sb[c][:, 2 * DIM + 512:3 * DIM],
                            start=(c == 0), stop=(c == KCH - 1))
                    nc.vector.tensor_copy(v_nat[t][:, 0:512], v_ps[:, 0, :])
                    nc.vector.tensor_copy(v_nat[t][:, 512:DIM],
                                          v_ps[:, 1, 0:256])
            # wp loads+casts
            for c in range(KCH):
                ws = spool.tile([128, DIM], F32, tag="wpstage",
                                name=f"wps{c}", bufs=2)
                nc.scalar.dma_start(out=ws,
                                    in_=wp_d[c * 128:(c + 1) * 128, :])
                nc.vector.tensor_copy(wp_sb[c], ws)
            # Q^T chunk 0
            with tc.tile_pool(name="ps_q0", bufs=1, space="PSUM") as ps_q0:
                mm_ps = ps_q0.tile([128, N], F32, tag="q_ps", name="q_ps")
                for c in range(KCH):
                    for g in range(N // 512):
                        nc.tensor.matmul(
                            mm_ps[:, g * 512:(g + 1) * 512],
                            wq_sb[c][:, 0:128],
                            xt[c][:, g * 512:(g + 1) * 512],
                            start=(c == 0), stop=(c == KCH - 1))
                nc.scalar.copy(qt[0], mm_ps)

    # deferred K^T/Q^T chunk closures (run inside attention stream,
    # borrowing the proj PSUM slot)
    def qk_backlog(ps):
        items = []
        for m in range(1, KCH):
            for lo, hi in [(0, 512), (512, NK)]:
                st = {}
                nsl = _nslices(hi - lo)

                def kstep(c, st=st, m=m, lo=lo, hi=hi, first=False):
                    if first:
                        st["ps"] = ps.tile([128, hi - lo], F32, tag="pr",
                                           bufs=1, name="kq_ps")
                    off = 0
                    for w in _nslices(hi - lo):
                        nc.tensor.matmul(
                            st["ps"][:, off:off + w],
                            wq_sb[c][:, DIM + m * 128:DIM + (m + 1) * 128],
                            xct[c][:, lo + off:lo + off + w],
                            start=(c == 0), stop=(c == KCH - 1))
                        off += w

                def fin_k(st=st, m=m, lo=lo, hi=hi):
                    nc.vector.tensor_copy(kt[m][:, lo:hi], st["ps"])

                if len(nsl) == 1:
                    # 1 matmul per c step: pair them up
                    items.append(lambda f=kstep: (f(0, first=True), f(1)))
                    items.append(lambda f=kstep: (f(2), f(3)))
                    items.append(lambda f=kstep: (f(4), f(5)))
                else:
                    for c in range(KCH):
                        items.append(
                            lambda c=c, f=kstep: f(c, first=(c == 0)))
                items.append(fin_k)
            for half in range(2):
                st = {}
                lo = half * 1024

                def qstep(c, st=st, m=m, lo=lo, first=False):
                    if first:
                        st["ps"] = ps.tile([128, 1024], F32, tag="pr",
                                           bufs=1, name="kq_ps")
                    for g in range(2):
                        nc.tensor.matmul(
                            st["ps"][:, g * 512:(g + 1) * 512],
                            wq_sb[c][:, m * 128:(m + 1) * 128],
                            xt[c][:, lo + g * 512:lo + (g + 1) * 512],
                            start=(c == 0), stop=(c == KCH - 1))

                def fin_q(st=st, m=m, lo=lo):
                    nc.vector.tensor_copy(qt[m][:, lo:lo + 1024], st["ps"])

                for c in range(KCH):
                    items.append(lambda c=c, f=qstep: f(c, first=(c == 0)))
                items.append(fin_q)
        return items

    # ---------------- attention + proj --------------------------
        _attention(nc, tc, qt, kt, v_nat, kb_t, ones_r, ot, nkc, wp_sb,
                   bp_bc, o_d, qk_backlog)
    qkvpool.release()
    cpool.release()


def _attention(nc, tc, qt, kt, v_nat, kb_t, ones_c, ot, nkc,
               wp, bp_bc, o_d, qk_backlog):
    with tc.tile_pool(name="p_sb", bufs=3) as ppool, \
         tc.tile_pool(name="rs_sb", bufs=2) as rspool, \
         tc.tile_pool(name="ep_sb", bufs=3) as eppool, \
         tc.tile_pool(name="out_sb", bufs=3) as outpool, \
         tc.tile_pool(name="dr_sb", bufs=3, space="DRAM") as drpool, \
         tc.tile_pool(name="ps_c", bufs=1, space="PSUM") as ps:

        backlog = qk_backlog(ps)

        def drain(k):
            for _ in range(min(k, len(backlog))):
                backlog.pop(0)()

        def emit_S(qh, hp, j):
            q0 = qh * QW
            s_t = ps.tile([128, 2, 512], F32, tag="s", bufs=2, name="s_t")
            for a in range(2):
                r0 = a * 64
                nc.tensor.matmul(
                    s_t[:, a, :],
                    kt[hp][r0:r0 + 64, j * 128:(j + 1) * 128],
                    qt[hp][r0:r0 + 64, q0:q0 + QW],
                    start=True, stop=True)
            return s_t

        def queue_proj(qh):
            def make_chunk(t_i, tag):
                st = {}

                def cstep(c, t_i=t_i, st=st, tag=tag):
                    if c == 0:
                        if tag == "s":
                            st["pr"] = ps.tile([128, 2, 512], F32, tag="s",
                                               bufs=2, name="pr")
                        else:
                            st["pr"] = ps.tile([128, 2, 512], F32,
                                               tag="pr", bufs=1, name="pr")
                    tl = (t_i % 4) * 128
                    pr = st["pr"]
                    nc.tensor.matmul(
                        pr[:, 0, :], ot[c][t_i // 4][:, tl:tl + 128],
                        wp[c][:, 0:512],
                        start=(c == 0), stop=(c == KCH - 1))
                    nc.tensor.matmul(
                        pr[:, 1, 0:256], ot[c][t_i // 4][:, tl:tl + 128],
                        wp[c][:, 512:DIM],
                        start=(c == 0), stop=(c == KCH - 1))

                def finish(t_i=t_i, st=st):
                    pr = st["pr"]
                    out_t = outpool.tile([128, DIM], F32, tag="out_t",
                                         name="out_t")
                    nc.vector.tensor_add(out_t[:, 0:512], pr[:, 0, :],
                                         bp_bc[:, 0:512])
                    nc.vector.tensor_add(out_t[:, 512:DIM], pr[:, 1, 0:256],
                                         bp_bc[:, 512:DIM])
                    nc.sync.dma_start(
                        out=o_d[t_i * 128:(t_i + 1) * 128, :], in_=out_t)

                return cstep, finish

            if qh < QH - 1:
                for ti in range(4):
                    cstep, finish = make_chunk(qh * 4 + ti, "pr")
                    for c in range(KCH):
                        backlog.append(lambda c=c, f=cstep: f(c))
                    backlog.append(finish)
            else:
                # c-major across 3 concurrent chunks, then the 4th
                chunks = [make_chunk(qh * 4 + ti,
                                     ("s", "s", "pr")[ti])
                          for ti in range(3)]
                for c in range(KCH):
                    for cstep, _ in chunks:
                        backlog.append(lambda c=c, f=cstep: f(c))
                for _, finish in chunks:
                    backlog.append(finish)
                cstep, finish = make_chunk(qh * 4 + 3, "s")
                for c in range(KCH):
                    backlog.append(lambda c=c, f=cstep: f(c))
                backlog.append(finish)

        def epilogue(qh, hp, rs_t, o_t):
            dn_t = ps.tile([128, 2, 512], F32, tag="s", bufs=2,
                           name="dn_t")
            for a in range(2):
                nc.tensor.matmul(dn_t[:, a, :], ones_c, rs_t[:, a, :],
                                 start=True, stop=True)
            dn_sb = eppool.tile([1, 2, 512], F32, tag="dn_sb", name="dn_sb")
            nc.vector.tensor_copy(dn_sb, dn_t[0:1, :, :])
            rc_dram = drpool.tile([1024], F32, tag="rc_dram", name="rc_dram")
            nc.sync.dma_start(out=rc_dram, in_=dn_sb)
            b_raw = eppool.tile([128, QW], F32, tag="b_raw", name="b_raw")
            for a in range(2):
                bc_ap = bass.AP(
                    tensor=rc_dram.tensor,
                    offset=rc_dram.offset + a * 512,
                    ap=[[0, 64], [1, 512]])
                nc.sync.dma_start(out=b_raw[a * 64:(a + 1) * 64, :],
                                  in_=bc_ap)
            rc_b = eppool.tile([128, QW], F32, tag="rc_b", name="rc_b")
            nc.vector.reciprocal_approx_fast(out=rc_b, in_=b_raw)
            nc.vector.tensor_mul(ot[hp][qh], o_t, rc_b)

        tri = [(qh, hp) for hp in range(H // 2) for qh in (0, 1, 2)]
        rest = [(3, hp) for hp in range(H // 2)]
        steps = [(qh, hp, j) for qh, hp in tri + rest
                 for j in range(nkc)]
        s_pend = {}
        s_pend[0] = emit_S(*steps[0])
        s_pend[1] = emit_S(*steps[1])
        hp_state = {}
        for idx, (qh, hp, j) in enumerate(steps):
            if j == 0:
                o_t = ps.tile([128, QW], F32, tag="o", bufs=2, name="o_t")
                rs_t = rspool.tile([128, 2, 512], BF16, tag="rs",
                                   name="rs_t")
                hp_state[(qh, hp)] = (o_t, rs_t)
            o_t, rs_t = hp_state[(qh, hp)]
            s_t = s_pend.pop(idx)
            pt_t = ppool.tile([128, 2, 512], BF16, tag="pt", name="pt_t")
            nc.scalar.activation(pt_t, s_t, Exp,
                                 bias=kb_t[:, j:j + 1], scale=SCALE)
            if idx + 2 < len(steps):
                s_pend[idx + 2] = emit_S(*steps[idx + 2])
            if j == 0:
                nc.vector.tensor_copy(rs_t, pt_t)
            else:
                nc.vector.tensor_add(rs_t, rs_t, pt_t)
            for a in range(2):
                h = 2 * hp + a
                nc.tensor.matmul(
                    o_t[a * 64:(a + 1) * 64, :],
                    v_nat[j][:, h * D:(h + 1) * D],
                    pt_t[:, a, :],
                    start=(j == 0), stop=(j == nkc - 1),
                    tile_position=(0, a * 64))
            if j == nkc - 1:
                epilogue(qh, hp, rs_t, o_t)
                del hp_state[(qh, hp)]
                if hp == H // 2 - 1:
                    queue_proj(qh)
            drain(2 if j == 0 else 1)
        drain(len(backlog))


_CACHE = {}


def _get_compiled(nkc):
    if nkc in _CACHE:
        return _CACHE[nkc]
    NK = nkc * 128
    nc = bacc.Bacc("TRN2", target_bir_lowering=False, debug=False,
                   num_devices=B)
    x_d = nc.dram_tensor("x", [N, DIM], F32, kind="ExternalInput").ap()
    ki_d = nc.dram_tensor("kidx", [NK], I32, kind="ExternalInput").ap()
    kb_d = nc.dram_tensor("kbias", [NK], F32, kind="ExternalInput").ap()
    wqkv_d = nc.dram_tensor("w_qkv", [DIM, 3 * DIM], F32,
                            kind="ExternalInput").ap()
    wp_d = nc.dram_tensor("w_proj", [DIM, DIM], F32,
                          kind="ExternalInput").ap()
    bp_d = nc.dram_tensor("b_proj", [DIM], F32, kind="ExternalInput").ap()
    o_d = nc.dram_tensor("out", [N, DIM], F32, kind="ExternalOutput").ap()
    with tile.TileContext(nc) as tc:
        _build(nc, tc, (x_d, ki_d, kb_d, wqkv_d, wp_d, bp_d, o_d), nkc)
    nc.compile()
    _CACHE[nkc] = nc
    return nc


def prep_run(x, mask, w_qkv, w_proj, b_proj):
    """Build the compiled program + per-core input maps."""
    x = np.ascontiguousarray(np.asarray(x, dtype=np.float32))
    mask = np.ascontiguousarray(np.asarray(mask, dtype=np.int32))
    w_qkv = np.ascontiguousarray(np.asarray(w_qkv, dtype=np.float32))
    w_proj = np.ascontiguousarray(np.asarray(w_proj, dtype=np.float32))
    b_proj = np.ascontiguousarray(np.asarray(b_proj, dtype=np.float32))

    idxs = [np.flatnonzero(mask[b]).astype(np.int32) for b in range(B)]
    max_valid = max(len(i) for i in idxs)
    nkc = min(NCH, max(1, -(-max_valid // 128)))
    NK = nkc * 128
    kidx = np.zeros((B, NK), dtype=np.int32)
    kbias = np.full((B, NK), -1.0e30, dtype=np.float32)
    for b in range(B):
        n = len(idxs[b])
        kidx[b, :n] = idxs[b]
        kbias[b, :n] = 0.0

    nc = _get_compiled(nkc)
    in_maps = [
        {"x": x[b], "kidx": kidx[b], "kbias": kbias[b], "w_qkv": w_qkv,
         "w_proj": w_proj, "b_proj": b_proj}
        for b in range(B)
    ]
    return nc, in_maps


def kernel(x, mask, w_qkv, w_proj, b_proj):
    nc, in_maps = prep_run(x, mask, w_qkv, w_proj, b_proj)
    last_err = None
    for _ in range(3):
        try:
            res = run_bass_kernel_spmd(nc, in_maps, list(range(B))).results
            return np.stack([res[b]["out"] for b in range(B)], axis=0)
        except Exception as e:  # transient device hiccup: retry
            last_err = e
    raise last_err
